# revision 1
# baseline (speedup 1.0000x reference)
"""Trainium2 Bass kernel for BoxMultiHeadedAttention (B=8, N=512, D=512, H=8).

Sharding: data-parallel over batch — each of the 8 NeuronCores computes one
batch element end-to-end; weights replicated; no collectives.

Per-core algorithm (transposed-attention layout [m(part), n(free)]):
  * q/k/v projections on PE (bf16) from DMA-transposed inputs.
  * scoresT = kT_h.T @ qT_h (1/8 folded into k); E = exp(scoresT + maskcol)
    on ACT (constant stability shift baked into maskcol).
  * geometry wg:
      - dx/dy: symmetric ln field on ACT; phase fractions
        t = (alpha_j/4pi) * dx2 replicated onto partitions by one-hot-scaled
        selector matmuls (exact f32), folded to [-1/2,1/2) by DVE
        magic-number round, then Sin on ACT (cos = sin(pi/2 - 2pi|f|));
        WG contraction on PE (bf16) with h-major output columns.
      - dw/dh: exactly separable (angle addition) -> rank-64 PE contraction
        of per-box sin/cos banks (phases folded the same way).
  * exp-domain softmax: T = E*(1 + obj_n*wgd), wgd = (max(wg+bG,1e-6)-1)*obj_m;
    row sums via PE ones-matmul; 1/s applied at AV eviction; final linear on
    PE from the transposed AV result.
"""
import math
import numpy as np
from contextlib import ExitStack

import concourse.bass as bass
import concourse.mybir as mybir
import concourse.tile as tile
from concourse.bass_utils import run_bass_kernel_spmd

F32 = mybir.dt.float32
BF16 = mybir.dt.bfloat16
AF = mybir.ActivationFunctionType
ALU = mybir.AluOpType

B, N, D, H = 8, 512, 512, 8
DK = D // H
P = 128
NRB = N // P
NG = 8
GM = 16
WAVE_LEN = 1000.0
MAGIC = 12582912.0
C2 = float(2.0 * math.log(0.001))
ESHIFT = -6.0
TWO_PI = float(2.0 * math.pi)
HALF_PI = float(math.pi / 2.0)
PI_ = float(math.pi)

_alphas = (100.0 / (WAVE_LEN ** (np.arange(8) / 8.0))).astype(np.float64)


def _split_multi_waits(nc):
    """walrus here accepts only ONE sync-wait per ISA instruction; hoist
    extras onto NoOps inserted before the offending instruction."""
    n_fix = 0
    for blk in nc.main_func.blocks:
        insts = list(blk.instructions)
        out, dirty = [], False
        for inst in insts:
            si = inst.sync_info
            waits = list(si.on_wait) if si is not None else []
            if len(waits) > 1:
                for kk, w in enumerate(waits[:-1]):
                    out.append(mybir.InstNoOp(
                        name=f"I-waitfix-{n_fix}-{kk}", engine=inst.engine,
                        sync_info=mybir.SyncInfo(on_wait=[w], on_update=[])))
                inst.sync_info = mybir.SyncInfo(
                    on_wait=[waits[-1]], on_update=list(si.on_update))
                n_fix += 1
                dirty = True
            out.append(inst)
        if dirty:
            blk.instructions = out
    return n_fix


def _selector_const():
    # SELAP[64*W + q*16 + m_loc, q, m_loc*8 + j] = alpha_j/(4pi)
    selap = np.zeros((P, 4, P), dtype=np.float32)
    for W in range(2):
        for q in range(4):
            for m_loc in range(GM):
                for j in range(8):
                    selap[64 * W + q * 16 + m_loc, q, m_loc * 8 + j] = \
                        _alphas[j] / (4.0 * math.pi)
    return selap


def _onehot8():
    # OH8[p, h, c] = 1.0 iff c == h  (lhsT column-one-hot for row sums)
    oh = np.zeros((P, H, H), dtype=np.float32)
    for h in range(H):
        oh[:, h, h] = 1.0
    return oh


def _wg_consts(WG, bG):
    out = {}
    # double-angle features: fsin_tile = sin(pi f)cos(pi f)  (weight 2*WGs),
    # fcos_tile = sin^2(pi f)                  (weight -2*WGc, const +WGc)
    gmap = [lambda j: j, lambda j: 32 + j, lambda j: 8 + j, lambda j: 40 + j]
    gscl = [2.0, -2.0, 2.0, -2.0]
    wblk = np.zeros((4, P, P), dtype=np.float32)
    for c in range(4):
        for m_loc in range(GM):
            for j in range(8):
                for h in range(H):
                    wblk[c, m_loc * 8 + j, h * GM + m_loc] = \
                        gscl[c] * WG[h, gmap[c](j)]
    out["WBLK"] = wblk

    acol = np.zeros((64, 1), np.float32)
    pcol_m = np.zeros((64, 1), np.float32)
    pcol_n = np.zeros((64, 1), np.float32)
    w1 = np.zeros((64, H), np.float32)
    for f in range(2):
        for j in range(8):
            gs = 16 + 8 * f + j
            gc = 48 + 8 * f + j
            a = _alphas[j] / (4.0 * math.pi)
            for t in range(4):
                k = (f * 8 + j) * 4 + t
                acol[k, 0] = a
                pcol_m[k, 0] = 0.25 if t in (0, 2) else 0.0
                if t == 0:
                    pcol_n[k, 0] = 0.0; w1[k] = WG[:, gs]
                elif t == 1:
                    pcol_n[k, 0] = 0.75; w1[k] = WG[:, gs]   # -cos -> +pi
                elif t == 2:
                    pcol_n[k, 0] = 0.25; w1[k] = WG[:, gc]
                else:
                    pcol_n[k, 0] = 0.0; w1[k] = WG[:, gc]
    out["ACOL"] = acol
    out["PCOL_M"], out["PCOL_N"] = pcol_m, pcol_n
    out["W1E"] = np.repeat(w1, GM, axis=1).astype(np.float32)
    # bG' = bG + sum_j (WGc_x + WGc_y)  (the "+1" of cos = 1 - 2 sin^2)
    bg2 = bG.astype(np.float64) + WG[:, 32:48].sum(axis=1)
    out["BGCOL"] = np.repeat(bg2.astype(np.float32), GM)[:, None]
    return out


def _host_prep(inputs):
    q = np.asarray(inputs["input_query"], np.float32)
    k = np.asarray(inputs["input_key"], np.float32)
    v = np.asarray(inputs["input_value"], np.float32)
    box = np.asarray(inputs["input_box"], np.float32)
    mask = np.asarray(inputs["mask"])
    nobj = np.asarray(inputs["not_objects"])
    WG = np.asarray(inputs["WG"], np.float32)
    bG = np.asarray(inputs["bG"], np.float32)
    wgc = _wg_consts(WG, bG)
    sela = _selector_const()

    x_min, y_min, x_max, y_max = [box[..., i] for i in range(4)]
    cx = (x_min + x_max) * 0.5
    cy = (y_min + y_max) * 0.5
    ww = x_max - x_min + 1.0
    hh = y_max - y_min + 1.0
    l2w = (2.0 * np.log(ww)).astype(np.float32)
    l2h = (2.0 * np.log(hh)).astype(np.float32)

    maskcol = (np.where(mask == 0, -1e9, 0.0) + ESHIFT).astype(np.float32)
    obj = (1.0 - nobj.astype(np.float32)).astype(np.float32)

    shared = {
        "Wq": np.asarray(inputs["Wq"], np.float32),
        "Wk": np.asarray(inputs["Wk"], np.float32),
        "Wv": np.asarray(inputs["Wv"], np.float32),
        "Wo": np.asarray(inputs["Wo"], np.float32),
        "bqcol": np.asarray(inputs["bq"], np.float32).reshape(NRB, P).T.copy(),
        "bk8col": (np.asarray(inputs["bk"], np.float32) * 8.0
                   ).reshape(NRB, P).T.copy(),
        "bvrow": np.asarray(inputs["bv"], np.float32),
        "borow": np.asarray(inputs["bo"], np.float32),
        "SELAP": sela, "IDENT": np.eye(P, dtype=np.float32),
        "ONEHOT8": _onehot8(),
        "WBLK": wgc["WBLK"], "W1E": wgc["W1E"],
        "BGCOL": wgc["BGCOL"], "ACOL": wgc["ACOL"],
        "PCOL_M": wgc["PCOL_M"], "PCOL_N": wgc["PCOL_N"],
    }
    in_maps = []
    for b in range(B):
        m = dict(shared)
        m.update({
            "xq": q[b].copy(), "xk": k[b].copy(), "xv": v[b].copy(),
            "cxrow": cx[b].copy(), "cyrow": cy[b].copy(),
            "cxcol": cx[b].reshape(NRB, P).T.copy(),
            "cycol": cy[b].reshape(NRB, P).T.copy(),
            "l2wrow": l2w[b].copy(), "l2hrow": l2h[b].copy(),
            "mcol": maskcol[b].reshape(NRB, P).T.copy(),
            "objrow": obj[b].copy(),
            "ocol": obj[b].reshape(NRB, P).T.copy(),
        })
        in_maps.append(m)
    return in_maps


def build_nc():
    nc = bass.Bass()

    def dp(name, shape):
        return nc.declare_dram_parameter(name, list(shape), F32, isOutput=False)

    xq = dp("xq", (N, D)); xk = dp("xk", (N, D)); xv = dp("xv", (N, D))
    Wq = dp("Wq", (D, D)); Wk = dp("Wk", (D, D)); Wv = dp("Wv", (D, D))
    Wo = dp("Wo", (D, D))
    bqcol = dp("bqcol", (P, NRB)); bk8col = dp("bk8col", (P, NRB))
    bvrow = dp("bvrow", (D,)); borow = dp("borow", (D,))
    cxrow = dp("cxrow", (N,)); cyrow = dp("cyrow", (N,))
    cxcol = dp("cxcol", (P, NRB)); cycol = dp("cycol", (P, NRB))
    l2wrow = dp("l2wrow", (N,)); l2hrow = dp("l2hrow", (N,))
    mcol = dp("mcol", (P, NRB)); objrow = dp("objrow", (N,))
    ocol = dp("ocol", (P, NRB))
    SELAP = dp("SELAP", (P, 4, P)); IDENT = dp("IDENT", (P, P))
    ONEHOT8 = dp("ONEHOT8", (P, H, H))
    WBLK = dp("WBLK", (4, P, P)); W1E = dp("W1E", (64, P))
    BGCOL = dp("BGCOL", (P, 1))
    ACOL = dp("ACOL", (64, 1))
    PCOL_M = dp("PCOL_M", (64, 1)); PCOL_N = dp("PCOL_N", (64, 1))
    out = nc.declare_dram_parameter("out", [N, D], F32, isOutput=True)
    rs_dram = nc.dram_tensor("rs_scratch", [H, N], F32)

    with ExitStack() as ctx:
        tc = ctx.enter_context(tile.TileContext(nc))
        const = ctx.enter_context(tc.tile_pool(name="const", bufs=1))
        persist = ctx.enter_context(tc.tile_pool(name="persist", bufs=1))

        # ---------------- constants ----------------
        with tc.tile_pool(name="cwork", bufs=2) as cwork:
            selap_f = cwork.tile([P, 4, P], F32, tag="selapf")
            nc.sync.dma_start(selap_f[:], SELAP[:])
            selap_t = const.tile([P, 4, P], F32, tag="selap")
            nc.vector.tensor_copy(selap_t[:], selap_f[:])
            oh8_f = cwork.tile([P, H, H], F32, tag="oh8f")
            nc.sync.dma_start(oh8_f[:], ONEHOT8[:])
            oh8_t = const.tile([P, H, H], BF16, tag="oh8")
            nc.vector.tensor_copy(oh8_t[:], oh8_f[:])
            ident_t = const.tile([P, P], F32, tag="ident")
            nc.sync.dma_start(ident_t[:], IDENT[:])
            wblk_t4 = []
            for c in range(4):
                wf = cwork.tile([P, P], F32, tag="wblkf")
                nc.sync.dma_start(wf[:], WBLK[c])
                wb = const.tile([P, P], BF16, tag=f"wblkb{c}")
                nc.vector.tensor_copy(wb[:], wf[:])
                wblk_t4.append(wb)
            w1e_f = const.tile([64, P], F32, tag="w1e")
            nc.sync.dma_start(w1e_f[:], W1E[:])
            bgcol_t = const.tile([P, 1], F32, tag="bgcol")
            nc.sync.dma_start(bgcol_t[:], BGCOL[:])
            bgm1_t = const.tile([P, 1], F32, tag="bgm1")
            nc.vector.tensor_scalar(bgm1_t[:], bgcol_t[:], -1.0, None, ALU.add)
            acol_t = const.tile([64, 1], F32, tag="acol")
            nc.sync.dma_start(acol_t[:], ACOL[:])
            pcolm_t = const.tile([64, 1], F32, tag="pcolm")
            nc.sync.dma_start(pcolm_t[:], PCOL_M[:])
            pcoln_t = const.tile([64, 1], F32, tag="pcoln")
            nc.sync.dma_start(pcoln_t[:], PCOL_N[:])
            ones_bf = const.tile([P, 1], BF16, tag="onesb")
            nc.vector.memset(ones_bf[:], 1.0)
            halfpi_t = const.tile([P, 1], F32, tag="halfpi")
            nc.vector.memset(halfpi_t[:], HALF_PI)
            mcol_t = const.tile([P, NRB], F32, tag="mcol")
            nc.sync.dma_start(mcol_t[:], mcol[:])
            bq_t = const.tile([P, NRB], F32, tag="bq")
            nc.sync.dma_start(bq_t[:], bqcol[:])
            bk8_t = const.tile([P, NRB], F32, tag="bk8")
            nc.sync.dma_start(bk8_t[:], bk8col[:])
            cxcol_t = const.tile([P, NRB], F32, tag="cxcol")
            nc.sync.dma_start(cxcol_t[:], cxcol[:])
            cycol_t = const.tile([P, NRB], F32, tag="cycol")
            nc.sync.dma_start(cycol_t[:], cycol[:])
            ocol_t = const.tile([P, NRB], F32, tag="ocol")
            nc.sync.dma_start(ocol_t[:], ocol[:])
            cxbc = const.tile([P, N], F32, tag="cxbc")
            nc.sync.dma_start(cxbc[:], cxrow[None, :].to_broadcast((P, N)))
            cybc = const.tile([P, N], F32, tag="cybc")
            nc.sync.dma_start(cybc[:], cyrow[None, :].to_broadcast((P, N)))
            l2wbc = const.tile([P, N], F32, tag="l2wbc")
            nc.sync.dma_start(l2wbc[:], l2wrow[None, :].to_broadcast((P, N)))
            l2hbc = const.tile([P, N], F32, tag="l2hbc")
            nc.sync.dma_start(l2hbc[:], l2hrow[None, :].to_broadcast((P, N)))
            objbc_f = cwork.tile([P, N], F32, tag="objbcf")
            nc.sync.dma_start(objbc_f[:], objrow[None, :].to_broadcast((P, N)))
            objbc = const.tile([P, N], BF16, tag="objbc")
            nc.vector.tensor_copy(objbc[:], objbc_f[:])
            bvbc = const.tile([P, D], F32, tag="bvbc")
            nc.sync.dma_start(bvbc[:], bvrow[None, :].to_broadcast((P, D)))
            bobc = const.tile([P, D], F32, tag="bobc")
            nc.sync.dma_start(bobc[:], borow[None, :].to_broadcast((P, D)))

        # ---------------- phase 1: transpose-load + projections ----------------
        xqTb = persist.tile([P, NRB, N], BF16, tag="xqTb")
        xkTb = persist.tile([P, NRB, N], BF16, tag="xkTb")
        xvTb = persist.tile([P, NRB, N], BF16, tag="xvTb")
        wq_b = persist.tile([P, NRB, D], BF16, tag="wqb")
        wk_b = persist.tile([P, NRB, D], BF16, tag="wkb")
        wv_b = persist.tile([P, NRB, D], BF16, tag="wvb")
        wo_b = persist.tile([P, NRB, D], BF16, tag="wob")
        qT = persist.tile([P, NRB, N], BF16, tag="qT")
        kTt = persist.tile([P, NRB, N], BF16, tag="kT")
        v_sb = persist.tile([P, NRB, D], BF16, tag="v_sb")

        with tc.tile_pool(name="work1", bufs=2) as work1, \
             tc.tile_pool(name="psum1", bufs=3, space="PSUM") as psum1:
            for (src, dstb) in ((xq, xqTb), (xk, xkTb), (xv, xvTb)):
                for rb in range(NRB):
                    xrb = work1.tile([P, D], F32, tag="xrb")
                    nc.sync.dma_start(xrb[:], src[rb * P:(rb + 1) * P, :])
                    for cb in range(NRB):
                        tp = psum1.tile([P, P], F32, tag="tp")
                        nc.tensor.transpose(tp[:], xrb[:, cb * P:(cb + 1) * P],
                                            ident_t[:])
                        nc.vector.tensor_copy(
                            dstb[:, cb, rb * P:(rb + 1) * P], tp[:])
            for (Wd, wb_) in ((Wq, wq_b), (Wk, wk_b), (Wv, wv_b), (Wo, wo_b)):
                wf = work1.tile([P, NRB, D], F32, tag="wldf")
                nc.sync.dma_start(wf[:],
                                  Wd.rearrange("(kb p) d -> p kb d", p=P))
                nc.vector.tensor_copy(wb_[:], wf[:])

            for (wb_, xb, dstT) in ((wq_b, xqTb, qT), (wk_b, xkTb, kTt)):
                for ob in range(NRB):
                    ps = psum1.tile([P, N], F32, tag="projps")
                    for kb in range(NRB):
                        nc.tensor.matmul(ps[:],
                                         wb_[:, kb, ob * P:(ob + 1) * P],
                                         xb[:, kb, :],
                                         start=(kb == 0),
                                         stop=(kb == NRB - 1))
                    if dstT is qT:
                        nc.vector.tensor_scalar(dstT[:, ob, :], ps[:],
                                                bq_t[:, ob:ob + 1], None,
                                                ALU.add)
                    else:
                        # kT = (ps + 8*bk) * 0.125
                        nc.vector.tensor_scalar(dstT[:, ob, :], ps[:],
                                                bk8_t[:, ob:ob + 1], 0.125,
                                                ALU.add, ALU.mult)
            for mb in range(NRB):
                ps = psum1.tile([P, D], F32, tag="projps")
                for kb in range(NRB):
                    nc.tensor.matmul(ps[:], xvTb[:, kb, mb * P:(mb + 1) * P],
                                     wv_b[:, kb, :],
                                     start=(kb == 0), stop=(kb == NRB - 1))
                vtmp = work1.tile([P, D], F32, tag="vev")
                nc.vector.tensor_tensor(vtmp[:], ps[:], bvbc[:], ALU.add)
                nc.vector.tensor_copy(v_sb[:, mb, :], vtmp[:])

        # ---------------- phase 2: ln fields ----------------
        dxy2 = persist.tile([P, NRB, 2, N], F32, tag="dxy2")
        with tc.tile_pool(name="work2", bufs=3) as work2:
            for rb in range(NRB):
                for (ci, cbc, ccol, l2bc) in ((0, cxbc, cxcol_t, l2wbc),
                                              (1, cybc, cycol_t, l2hbc)):
                    d_ = work2.tile([P, N], F32, tag="geo_d")
                    nc.vector.tensor_scalar(d_[:], cbc[:], ccol[:, rb:rb + 1],
                                            None, ALU.subtract)
                    d2 = work2.tile([P, N], F32, tag="geo_d2")
                    nc.vector.tensor_tensor(d2[:], d_[:], d_[:], ALU.mult)
                    l2t = work2.tile([P, N], F32, tag="geo_l2")
                    nc.scalar.activation(l2t[:], d2[:], AF.Ln)
                    g_ = work2.tile([P, N], F32, tag="geo_g")
                    nc.vector.tensor_tensor(g_[:], l2t[:], l2bc[:],
                                            ALU.subtract)
                    nc.vector.tensor_scalar_max(dxy2[:, rb, ci, :], g_[:], C2)

        # ---------------- phase 3: dw/dh banks ----------------
        bankM = persist.tile([64, N], BF16, tag="bankM")
        bankN = persist.tile([64, N], BF16, tag="bankN")
        with tc.tile_pool(name="work3", bufs=2) as work3:
            for (pcol, bank) in ((pcolm_t, bankM), (pcoln_t, bankN)):
                t_ = work3.tile([64, N], F32, tag="bk_t")
                nc.vector.tensor_scalar(t_[:32, :], l2wbc[:32, :],
                                        acol_t[:32, :], pcol[:32, :],
                                        ALU.mult, ALU.add)
                nc.vector.tensor_scalar(t_[32:, :], l2hbc[32:64, :],
                                        acol_t[32:, :], pcol[32:, :],
                                        ALU.mult, ALU.add)
                r_ = work3.tile([64, N], F32, tag="bk_r")
                nc.vector.tensor_scalar(r_[:], t_[:], MAGIC, -MAGIC,
                                        ALU.add, ALU.add)
                f_ = work3.tile([64, N], F32, tag="bk_f")
                nc.vector.tensor_tensor(f_[:], t_[:], r_[:], ALU.subtract)
                nc.scalar.activation(bank[:], f_[:], AF.Sin, scale=TWO_PI)

        # ---------------- phase 4: wg ----------------
        wgdT = persist.tile([P, H, NRB, N], BF16, tag="wgdT")
        with tc.tile_pool(name="work4", bufs=3) as work4, \
             tc.tile_pool(name="psum_u", bufs=2, space="PSUM") as psum_u, \
             tc.tile_pool(name="psum_wg", bufs=3, space="PSUM") as psum_wg:
            for rb in range(NRB):
                wgd_il = work4.tile([P, NG, N], BF16, tag="wgd_il")
                for g in range(NG):
                    lhs_wh = work4.tile([64, P], BF16, tag="lhs_wh")
                    mbase = rb * P + g * GM
                    nc.vector.tensor_tensor(
                        lhs_wh[:].rearrange("k (h m) -> k h m", h=H),
                        w1e_f[:].rearrange("k (h m) -> k h m", h=H),
                        bankM[:, mbase:mbase + GM][:, None, :]
                            .to_broadcast((64, H, GM)),
                        ALU.mult)
                    ups = psum_u.tile([P, 2, N], F32, tag="ups")
                    off = 64 * (g // 4)
                    q = g % 4
                    for ci in range(2):
                        nc.tensor.matmul(ups[:, ci, :],
                                         selap_t[off:off + 64, q, :],
                                         dxy2[off:off + 64, rb, ci, :],
                                         start=True, stop=True)
                    rr = work4.tile([P, 2, N], F32, tag="fold_r")
                    nc.vector.tensor_scalar(rr[:], ups[:], MAGIC, -MAGIC,
                                            ALU.add, ALU.add)
                    ff = work4.tile([P, 2, N], F32, tag="fold_f")
                    nc.vector.tensor_tensor(ff[:], ups[:], rr[:],
                                            ALU.subtract)
                    # half-angle: s2 = sin(pi f), c2 = cos(pi f) = sin(pi/2-pi f)
                    s2 = work4.tile([P, 2, N], BF16, tag="s2")
                    nc.scalar.activation(s2[:], ff[:], AF.Sin, scale=PI_)
                    c2 = work4.tile([P, 2, N], BF16, tag="c2")
                    nc.scalar.activation(c2[:], ff[:], AF.Sin, scale=-PI_,
                                         bias=halfpi_t[:])
                    fsin = work4.tile([P, 2, N], BF16, tag="fsin")
                    nc.vector.tensor_tensor(fsin[:], s2[:], c2[:], ALU.mult)
                    fcos = work4.tile([P, 2, N], BF16, tag="fcos")
                    nc.vector.tensor_tensor(fcos[:], s2[:], s2[:], ALU.mult)
                    wgp = psum_wg.tile([P, N], F32, tag="wgp")
                    nc.tensor.matmul(wgp[:], wblk_t4[0][:], fsin[:, 0, :],
                                     start=True, stop=False)
                    nc.tensor.matmul(wgp[:], wblk_t4[1][:], fcos[:, 0, :],
                                     start=False, stop=False)
                    nc.tensor.matmul(wgp[:], wblk_t4[2][:], fsin[:, 1, :],
                                     start=False, stop=False)
                    nc.tensor.matmul(wgp[:], wblk_t4[3][:], fcos[:, 1, :],
                                     start=False, stop=False)
                    nc.tensor.matmul(wgp[:], lhs_wh[:], bankN[:],
                                     start=False, stop=True)
                    # wgd = max(wg + bG', 1e-6) - 1 = max(wg + bG'-1, 1e-6-1)
                    nc.vector.tensor_scalar(wgd_il[:, g, :], wgp[:],
                                            bgm1_t[:], 1e-6 - 1.0,
                                            ALU.add, ALU.max)
                for h in range(H):
                    for g in range(NG):
                        nc.sync.dma_start(
                            wgdT[g * GM:(g + 1) * GM, h, rb, :],
                            wgd_il[h * GM:(h + 1) * GM, g, :])

        # ---------------- phase 5: attention ----------------
        ot = persist.tile([P, NRB, N], BF16, tag="ot")
        with tc.tile_pool(name="work5", bufs=3) as work5, \
             tc.tile_pool(name="psum5", bufs=2, space="PSUM") as psum5, \
             tc.tile_pool(name="psum_s", bufs=2, space="PSUM") as psum_s, \
             tc.tile_pool(name="psum_av", bufs=2, space="PSUM") as psum_av:

            objpair = persist.tile([P, NRB, N], BF16, tag="objpair")
            for rb in range(NRB):
                nc.vector.tensor_scalar(objpair[:, rb, :], objbc[:],
                                        ocol_t[:, rb:rb + 1], None, ALU.mult)
            # head PAIRS (2k, 2k+1) share kT/qT block ob=k at offsets 0/64:
            # one exp + one combine chain over [P, 2, N], shared av bank.
            for ob in range(H // 2):
                h0 = 2 * ob
                av = psum_av.tile([P, N], F32, tag="avps")
                sbank = psum_s.tile([H, N], F32, tag="sbank")
                for rb in range(NRB):
                    st2 = psum5.tile([P, 2, N], F32, tag="stps")
                    for hi in range(2):
                        po = hi * DK
                        nc.tensor.matmul(
                            st2[:, hi, :],
                            kTt[po:po + DK, ob, rb * P:(rb + 1) * P],
                            qT[po:po + DK, ob, :], start=True, stop=True)
                    e_ = work5.tile([P, 2, N], BF16, tag="e_t")
                    nc.scalar.activation(e_[:], st2[:], AF.Exp,
                                         bias=mcol_t[:, rb:rb + 1])
                    e1 = work5.tile([P, 2, N], BF16, tag="e1_t")
                    nc.vector.tensor_tensor(
                        e1[:], e_[:],
                        objpair[:, rb, None, :].to_broadcast((P, 2, N)),
                        ALU.mult)
                    e2 = work5.tile([P, 2, N], BF16, tag="e2_t")
                    nc.vector.tensor_tensor(e2[:], e1[:],
                                            wgdT[:, h0:h0 + 2, rb, :],
                                            ALU.mult)
                    tt_ = work5.tile([P, 2, N], BF16, tag="tt_t")
                    nc.vector.tensor_tensor(tt_[:], e_[:], e2[:], ALU.add)
                    for hi in range(2):
                        po = hi * DK
                        nc.tensor.matmul(sbank[:], oh8_t[:, h0 + hi, :],
                                         tt_[:, hi, :],
                                         start=(rb == 0 and hi == 0),
                                         stop=(rb == NRB - 1 and hi == 1),
                                         skip_group_check=True)
                        nc.tensor.matmul(av[po:po + DK, :],
                                         v_sb[:, rb,
                                              (h0 + hi) * DK:(h0 + hi + 1) * DK],
                                         tt_[:, hi, :], start=(rb == 0),
                                         stop=(rb == NRB - 1),
                                         skip_group_check=True)
                rs = work5.tile([H, N], F32, tag="rs")
                nc.vector.reciprocal(rs[:], sbank[:])
                nc.sync.dma_start(rs_dram[h0:h0 + 2, :], rs[h0:h0 + 2, :])
                rr_b = work5.tile([P, N], F32, tag="rr_b")
                for hi in range(2):
                    nc.sync.dma_start(
                        rr_b[hi * DK:(hi + 1) * DK, :],
                        rs_dram[h0 + hi:h0 + hi + 1, :].to_broadcast((DK, N)))
                nc.vector.tensor_tensor(ot[:, ob, :], av[:], rr_b[:], ALU.mult)

        # final projection: out[n, d]  (own PSUM scope)
        with tc.tile_pool(name="work6", bufs=2) as work6, \
             tc.tile_pool(name="psum6", bufs=2, space="PSUM") as psum6:
            for r in range(NRB):
                ps = psum6.tile([P, D], F32, tag="fps")
                for kt in range(NRB):
                    nc.tensor.matmul(ps[:], ot[:, kt, r * P:(r + 1) * P],
                                     wo_b[:, kt, :],
                                     start=(kt == 0), stop=(kt == NRB - 1))
                fo = work6.tile([P, D], F32, tag="fo")
                nc.vector.tensor_tensor(fo[:], ps[:], bobc[:], ALU.add)
                nc.sync.dma_start(out[r * P:(r + 1) * P, :], fo[:])

    _split_multi_waits(nc)
    return nc


_NC_CACHE = {}


def kernel(**inputs):
    in_maps = _host_prep(inputs)
    if "nc" not in _NC_CACHE:
        _NC_CACHE["nc"] = build_nc()
    nc = _NC_CACHE["nc"]
    res = run_bass_kernel_spmd(nc, in_maps, list(range(B)))
    out = np.stack([res.results[b]["out"] for b in range(B)], axis=0)
    return out.astype(np.float32)


if __name__ == "__main__":
    print("kernel module ok")



# revision 12
# speedup vs baseline: 1.3454x; 1.3454x over previous
"""Trainium2 Bass kernel for BoxMultiHeadedAttention (B=8, N=512, D=512, H=8).

Sharding: data-parallel over batch — each of the 8 NeuronCores computes one
batch element end-to-end; weights replicated; no collectives.

v2 layout/engine plan (vs v1):
  * inputs and weights are transposed + bf16-converted on the host, so the
    on-chip transpose pass disappears; projections read them directly.
  * constants are packed into a handful of wide DMAs.
  * geometry wg: selector matmul merged over ci ([P,2,N] f32); magic-round
    fold on DVE; Sin pair on ACT; double-angle products on DVE (bf16 4x);
    PSUM eviction on ACT as wgdR = Relu(wg + bG' - eps), with the
    "-1+eps" constant folded into a precomputed objc tile.
  * the h-major -> m-major wgd shuffle goes through a DRAM bounce:
    8 per-h writes per rb + 1 read per rb (36 DMAs total) instead of 256
    SBUF->SBUF DMAs.
  * ph5: uu = objc + objpair*wgdR precomputed per (rb, head-pair) as soon
    as the bounce read lands; inner chain is st2 (PE) -> Exp (ACT) ->
    tt = e*uu (DVE) -> AV/rowsum (PE). Softmax 1/s is broadcast across
    partitions with a one-hot matmul instead of a DRAM roundtrip.
"""
import math
import numpy as np
from contextlib import ExitStack

import concourse.bass as bass
import concourse.mybir as mybir
import concourse.tile as tile
from concourse.bass_utils import run_bass_kernel_spmd

F32 = mybir.dt.float32
BF16 = mybir.dt.bfloat16
AF = mybir.ActivationFunctionType
ALU = mybir.AluOpType

B, N, D, H = 8, 512, 512, 8
DK = D // H
P = 128
NRB = N // P
NG = 8
GM = 16
NPAIR = H // 2
WAVE_LEN = 1000.0
MAGIC = 12582912.0
C2 = float(2.0 * math.log(0.001))
ESHIFT = -6.0
CM1 = 1e-6 - 1.0
TWO_PI = float(2.0 * math.pi)
HALF_PI = float(math.pi / 2.0)
PI_ = float(math.pi)

_alphas = (100.0 / (WAVE_LEN ** (np.arange(8) / 8.0))).astype(np.float64)

# column indices in colpack
C_BQ, C_BK, C_MC, C_CX, C_CY, C_NCX, C_NCY, C_OC = 0, 4, 8, 12, 16, 20, 24, 28
C_BGR, C_HPI = 32, 33
NCOL = 34
# row indices in rowpack
R_CX, R_CY, R_LW, R_LH, R_OBJ, R_BV, R_BO = range(7)
NROW = 7


def _split_multi_waits(nc):
    """walrus here accepts only ONE sync-wait per ISA instruction; hoist
    extras onto NoOps inserted before the offending instruction."""
    n_fix = 0
    for blk in nc.main_func.blocks:
        insts = list(blk.instructions)
        out, dirty = [], False
        for inst in insts:
            si = inst.sync_info
            waits = list(si.on_wait) if si is not None else []
            if len(waits) > 1:
                for kk, w in enumerate(waits[:-1]):
                    out.append(mybir.InstNoOp(
                        name=f"I-waitfix-{n_fix}-{kk}", engine=inst.engine,
                        sync_info=mybir.SyncInfo(on_wait=[w], on_update=[])))
                inst.sync_info = mybir.SyncInfo(
                    on_wait=[waits[-1]], on_update=list(si.on_update))
                n_fix += 1
                dirty = True
            out.append(inst)
        if dirty:
            blk.instructions = out
    return n_fix


def _selector_const():
    # SELAP[64*W + q*16 + m_loc, q, m_loc*8 + j] = alpha_j/(4pi)
    selap = np.zeros((P, 4, P), dtype=np.float32)
    for W in range(2):
        for q in range(4):
            for m_loc in range(GM):
                for j in range(8):
                    selap[64 * W + q * 16 + m_loc, q, m_loc * 8 + j] = \
                        _alphas[j] / (4.0 * math.pi)
    return selap


def _onehot2():
    # OH2[p, hi, c] = 1.0 iff c == hi  (lhsT column-one-hot for row sums)
    oh = np.zeros((P, 2, 2), dtype=np.float32)
    for hi in range(2):
        oh[:, hi, hi] = 1.0
    return oh


def _selpair():
    # SELP[k(hi of 2), (hi',dk) col] = 1 iff hi == hi'
    sp = np.zeros((2, P), dtype=np.float32)
    for hi in range(2):
        sp[hi, hi * DK:(hi + 1) * DK] = 1.0
    return sp


def _wg_consts(WG, bG):
    out = {}
    # double-angle features: fsin_tile = sin(pi f)cos(pi f)  (weight 2*WGs),
    # fcos_tile = sin^2(pi f)                  (weight -2*WGc, const +WGc)
    gmap = [lambda j: j, lambda j: 32 + j, lambda j: 8 + j, lambda j: 40 + j]
    gscl = [2.0, -2.0, 2.0, -2.0]
    wblk = np.zeros((4, P, P), dtype=np.float32)
    for c in range(4):
        for m_loc in range(GM):
            for j in range(8):
                for h in range(H):
                    wblk[c, m_loc * 8 + j, h * GM + m_loc] = \
                        gscl[c] * WG[h, gmap[c](j)]
    out["WBLK"] = wblk.transpose(1, 0, 2).copy()  # [P, 4, P]

    acol = np.zeros((64, 1), np.float32)
    pcol_m = np.zeros((64, 1), np.float32)
    pcol_n = np.zeros((64, 1), np.float32)
    w1 = np.zeros((64, H), np.float32)
    for f in range(2):
        for j in range(8):
            gs = 16 + 8 * f + j
            gc = 48 + 8 * f + j
            a = _alphas[j] / (4.0 * math.pi)
            for t in range(4):
                k = (f * 8 + j) * 4 + t
                acol[k, 0] = a
                pcol_m[k, 0] = 0.25 if t in (0, 2) else 0.0
                if t == 0:
                    pcol_n[k, 0] = 0.0; w1[k] = WG[:, gs]
                elif t == 1:
                    pcol_n[k, 0] = 0.75; w1[k] = WG[:, gs]   # -cos -> +pi
                elif t == 2:
                    pcol_n[k, 0] = 0.25; w1[k] = WG[:, gc]
                else:
                    pcol_n[k, 0] = 0.0; w1[k] = WG[:, gc]
    out["ACOL"] = acol
    out["PCOL_M"], out["PCOL_N"] = pcol_m, pcol_n
    out["W1E"] = np.repeat(w1, GM, axis=1).astype(np.float32)
    # bG' = bG + sum_j (WGc_x + WGc_y)  (the "+1" of cos = 1 - 2 sin^2)
    bg2 = bG.astype(np.float64) + WG[:, 32:48].sum(axis=1)
    out["BGR"] = np.repeat((bg2 - 1e-6).astype(np.float32), GM)
    return out


def _to_bf16(a):
    import ml_dtypes
    return a.astype(ml_dtypes.bfloat16)


def _host_prep(inputs):
    q = np.asarray(inputs["input_query"], np.float32)
    k = np.asarray(inputs["input_key"], np.float32)
    v = np.asarray(inputs["input_value"], np.float32)
    box = np.asarray(inputs["input_box"], np.float32)
    mask = np.asarray(inputs["mask"])
    nobj = np.asarray(inputs["not_objects"])
    WG = np.asarray(inputs["WG"], np.float32)
    bG = np.asarray(inputs["bG"], np.float32)
    wgc = _wg_consts(WG, bG)
    sela = _selector_const()

    x_min, y_min, x_max, y_max = [box[..., i] for i in range(4)]
    cx = (x_min + x_max) * 0.5
    cy = (y_min + y_max) * 0.5
    ww = x_max - x_min + 1.0
    hh = y_max - y_min + 1.0
    l2w = (2.0 * np.log(ww)).astype(np.float32)
    l2h = (2.0 * np.log(hh)).astype(np.float32)

    maskcol = (np.where(mask == 0, -1e9, 0.0) + ESHIFT).astype(np.float32)
    obj = (1.0 - nobj.astype(np.float32)).astype(np.float32)

    def wtile(W, scale=1.0):
        # [D, D] -> [P, NRB, D] bf16 with (kb p) d -> p kb d
        return _to_bf16((np.asarray(W, np.float32) * scale)
                        .reshape(NRB, P, D).transpose(1, 0, 2).copy())

    def xtile(x):
        # [N, D] -> xT [P, NRB, N] bf16 with (kb p) n -> p kb n
        return _to_bf16(x.T.reshape(NRB, P, N).transpose(1, 0, 2).copy())

    w64 = np.zeros((64, 131), np.float32)
    w64[:, :128] = wgc["W1E"]
    w64[:, 128] = wgc["ACOL"][:, 0]
    w64[:, 129] = wgc["PCOL_M"][:, 0]
    w64[:, 130] = wgc["PCOL_N"][:, 0]

    shared = {
        "Wqb": wtile(inputs["Wq"]),
        "Wkb": wtile(inputs["Wk"], 0.125),
        "Wvb": wtile(inputs["Wv"]),
        "Wob": wtile(inputs["Wo"]),
        "SELAP": sela,
        "WBLK": wgc["WBLK"],
        "W64": w64,
        "OH2": _onehot2(),
        "SELP": _selpair(),
    }
    in_maps = []
    for b in range(B):
        colpack = np.zeros((P, NCOL), np.float32)
        colpack[:, C_BQ:C_BQ + 4] = np.asarray(inputs["bq"], np.float32) \
            .reshape(NRB, P).T
        colpack[:, C_BK:C_BK + 4] = (np.asarray(inputs["bk"], np.float32)
                                     * 0.125).reshape(NRB, P).T
        colpack[:, C_MC:C_MC + 4] = maskcol[b].reshape(NRB, P).T
        colpack[:, C_CX:C_CX + 4] = cx[b].reshape(NRB, P).T
        colpack[:, C_CY:C_CY + 4] = cy[b].reshape(NRB, P).T
        colpack[:, C_NCX:C_NCX + 4] = -cx[b].reshape(NRB, P).T
        colpack[:, C_NCY:C_NCY + 4] = -cy[b].reshape(NRB, P).T
        colpack[:, C_OC:C_OC + 4] = obj[b].reshape(NRB, P).T
        colpack[:, C_BGR] = wgc["BGR"]
        colpack[:, C_HPI] = HALF_PI

        rowpack = np.zeros((NROW, N), np.float32)
        rowpack[R_CX] = cx[b]
        rowpack[R_CY] = cy[b]
        rowpack[R_LW] = l2w[b]
        rowpack[R_LH] = l2h[b]
        rowpack[R_OBJ] = obj[b]
        rowpack[R_BV] = np.asarray(inputs["bv"], np.float32)
        rowpack[R_BO] = np.asarray(inputs["bo"], np.float32)

        m = dict(shared)
        m.update({
            "xqT": xtile(q[b]), "xkT": xtile(k[b]), "xvT": xtile(v[b]),
            "colpack": colpack, "rowpack": rowpack,
        })
        in_maps.append(m)
    return in_maps


def build_nc():
    nc = bass.Bass()

    def dp(name, shape, dt=F32):
        return nc.declare_dram_parameter(name, list(shape), dt, isOutput=False)

    xqT = dp("xqT", (P, NRB, N), BF16)
    xkT = dp("xkT", (P, NRB, N), BF16)
    xvT = dp("xvT", (P, NRB, N), BF16)
    Wqb = dp("Wqb", (P, NRB, D), BF16)
    Wkb = dp("Wkb", (P, NRB, D), BF16)
    Wvb = dp("Wvb", (P, NRB, D), BF16)
    Wob = dp("Wob", (P, NRB, D), BF16)
    colpack = dp("colpack", (P, NCOL))
    rowpack = dp("rowpack", (NROW, N))
    SELAP = dp("SELAP", (P, 4, P))
    WBLK = dp("WBLK", (P, 4, P))
    W64 = dp("W64", (64, 131))
    OH2 = dp("OH2", (P, 2, 2))
    SELP = dp("SELP", (2, P))
    out = nc.declare_dram_parameter("out", [N, D], F32, isOutput=True)
    # bounce scratch: [rb][g][m_loc][h][n] bf16
    wgdd = nc.dram_tensor("wgdd", [NRB, NG, GM, H, N], BF16)

    with ExitStack() as ctx:
        tc = ctx.enter_context(tile.TileContext(nc))
        const = ctx.enter_context(tc.tile_pool(name="const", bufs=1))
        persist = ctx.enter_context(tc.tile_pool(name="persist", bufs=1))

        # ---------------- constants ----------------
        col_t = const.tile([P, NCOL], F32, tag="colpk")
        nc.sync.dma_start(col_t[:], colpack[:])
        rowbc = const.tile([P, NROW, N], F32, tag="rowpk")
        nc.sync.dma_start(
            rowbc[:], rowpack[None, :, :].to_broadcast((P, NROW, N)))
        selap_t = const.tile([P, 4, P], F32, tag="selap")
        nc.sync.dma_start(selap_t[:], SELAP[:])
        wblk_f = const.tile([P, 4, P], F32, tag="wblkf")
        nc.sync.dma_start(wblk_f[:], WBLK[:])
        wblk_b = const.tile([P, 4, P], BF16, tag="wblkb")
        nc.gpsimd.tensor_copy(wblk_b[:], wblk_f[:])
        w64_t = const.tile([64, 131], F32, tag="w64")
        nc.sync.dma_start(w64_t[:], W64[:])
        oh2_f = const.tile([P, 2, 2], F32, tag="oh2f")
        nc.sync.dma_start(oh2_f[:], OH2[:])
        oh2_t = const.tile([P, 2, 2], BF16, tag="oh2")
        nc.gpsimd.tensor_copy(oh2_t[:], oh2_f[:])
        selp_f = const.tile([2, P], F32, tag="selpf")
        nc.sync.dma_start(selp_f[:], SELP[:])
        objbc = const.tile([P, N], BF16, tag="objbc")
        nc.gpsimd.tensor_copy(objbc[:], rowbc[:, R_OBJ, :])

        w1e_f = w64_t[:, 0:128]
        acol_t = w64_t[:, 128:129]
        pcolm_t = w64_t[:, 129:130]
        pcoln_t = w64_t[:, 130:131]

        # objpair/objc per rb: objpair = objbc * ocol; objc = 1 + CM1*objpair
        objpair = persist.tile([P, NRB, N], BF16, tag="objpair")
        objc = persist.tile([P, NRB, N], BF16, tag="objc")
        for rb in range(NRB):
            nc.vector.tensor_scalar(objpair[:, rb, :], objbc[:],
                                    col_t[:, C_OC + rb:C_OC + rb + 1], None,
                                    ALU.mult)
            nc.vector.tensor_scalar(objc[:, rb, :], objpair[:, rb, :],
                                    CM1, 1.0, ALU.mult, ALU.add)

        # ---------------- phase 1: projections ----------------
        xqb = persist.tile([P, NRB, N], BF16, tag="xqb")
        nc.sync.dma_start(xqb[:], xqT[:])
        xkb = persist.tile([P, NRB, N], BF16, tag="xkb")
        nc.sync.dma_start(xkb[:], xkT[:])
        xvb = persist.tile([P, NRB, N], BF16, tag="xvb")
        nc.sync.dma_start(xvb[:], xvT[:])
        wq_b = persist.tile([P, NRB, D], BF16, tag="wqb")
        nc.sync.dma_start(wq_b[:], Wqb[:])
        wk_b = persist.tile([P, NRB, D], BF16, tag="wkb")
        nc.sync.dma_start(wk_b[:], Wkb[:])
        wv_b = persist.tile([P, NRB, D], BF16, tag="wvb")
        nc.sync.dma_start(wv_b[:], Wvb[:])
        wo_b = persist.tile([P, NRB, D], BF16, tag="wob")
        nc.sync.dma_start(wo_b[:], Wob[:])

        qT = persist.tile([P, NRB, N], BF16, tag="qT")
        kTt = persist.tile([P, NRB, N], BF16, tag="kT")
        v_sb = persist.tile([P, NRB, D], BF16, tag="v_sb")

        with tc.tile_pool(name="psum1", bufs=3, space="PSUM") as psum1:
            for (wb_, xb, dstT, bcol) in ((wq_b, xqb, qT, C_BQ),
                                          (wk_b, xkb, kTt, C_BK)):
                for ob in range(NRB):
                    ps = psum1.tile([P, N], F32, tag="projps")
                    for kb in range(NRB):
                        nc.tensor.matmul(ps[:],
                                         wb_[:, kb, ob * P:(ob + 1) * P],
                                         xb[:, kb, :],
                                         start=(kb == 0),
                                         stop=(kb == NRB - 1))
                    nc.vector.tensor_scalar(dstT[:, ob, :], ps[:],
                                            col_t[:, bcol + ob:bcol + ob + 1],
                                            None, ALU.add)
            for mb in range(NRB):
                ps = psum1.tile([P, D], F32, tag="projps")
                for kb in range(NRB):
                    nc.tensor.matmul(ps[:], xvb[:, kb, mb * P:(mb + 1) * P],
                                     wv_b[:, kb, :],
                                     start=(kb == 0), stop=(kb == NRB - 1))
                nc.vector.tensor_tensor(v_sb[:, mb, :], ps[:],
                                        rowbc[:, R_BV, :], ALU.add)

        # ---------------- phase 2: ln fields ----------------
        dxy2 = persist.tile([P, NRB, 2, N], F32, tag="dxy2")
        with tc.tile_pool(name="work2", bufs=3) as work2:
            for rb in range(NRB):
                for (ci, rbc, ncc) in ((0, R_CX, C_NCX), (1, R_CY, C_NCY)):
                    d2 = work2.tile([P, N], F32, tag="geo_d2")
                    nc.scalar.activation(d2[:], rowbc[:, rbc, :], AF.Square,
                                         bias=col_t[:, ncc + rb:ncc + rb + 1])
                    l2t = work2.tile([P, N], F32, tag="geo_l2")
                    nc.scalar.activation(l2t[:], d2[:], AF.Ln)
                    g_ = work2.tile([P, N], F32, tag="geo_g")
                    nc.gpsimd.tensor_tensor(
                        g_[:], l2t[:], rowbc[:, R_LW + ci, :], ALU.subtract)
                    nc.gpsimd.tensor_scalar(dxy2[:, rb, ci, :], g_[:],
                                            C2, None, ALU.max)

        # ---------------- phase 3: dw/dh banks ----------------
        bankM = persist.tile([64, N], BF16, tag="bankM")
        bankN = persist.tile([64, N], BF16, tag="bankN")
        with tc.tile_pool(name="work3", bufs=2) as work3:
            for (pcol, bank) in ((pcolm_t, bankM), (pcoln_t, bankN)):
                t_ = work3.tile([64, N], F32, tag="bk_t")
                nc.vector.tensor_scalar(t_[:32, :], rowbc[:32, R_LW, :],
                                        acol_t[:32, :], pcol[:32, :],
                                        ALU.mult, ALU.add)
                nc.vector.tensor_scalar(t_[32:, :], rowbc[32:64, R_LH, :],
                                        acol_t[32:, :], pcol[32:, :],
                                        ALU.mult, ALU.add)
                r_ = work3.tile([64, N], F32, tag="bk_r")
                nc.vector.tensor_scalar(r_[:], t_[:], MAGIC, -MAGIC,
                                        ALU.add, ALU.add)
                f_ = work3.tile([64, N], F32, tag="bk_f")
                nc.vector.tensor_tensor(f_[:], t_[:], r_[:], ALU.subtract)
                nc.scalar.activation(bank[:], f_[:], AF.Sin, scale=TWO_PI)

        # ---------------- phase 4: wg + bounce ----------------
        with tc.tile_pool(name="work4", bufs=3) as work4, \
             tc.tile_pool(name="ilpool", bufs=2) as ilpool, \
             tc.tile_pool(name="psum_u", bufs=2, space="PSUM") as psum_u, \
             tc.tile_pool(name="psum_wg", bufs=2, space="PSUM") as psum_wg:
            for rb in range(NRB):
                wgd_il = ilpool.tile([P, NG, N], BF16, tag="wgd_il")
                for g in range(NG):
                    lhs_wh = work4.tile([64, P], BF16, tag="lhs_wh")
                    mbase = rb * P + g * GM
                    nc.gpsimd.tensor_tensor(
                        lhs_wh[:].rearrange("k (h m) -> k h m", h=H),
                        w1e_f.rearrange("k (h m) -> k h m", h=H),
                        bankM[:, mbase:mbase + GM][:, None, :]
                            .to_broadcast((64, H, GM)),
                        ALU.mult)
                    ups = psum_u.tile([P, 2, N], F32, tag="ups")
                    off = 64 * (g // 4)
                    qq = g % 4
                    for ci in range(2):
                        nc.tensor.matmul(ups[:, ci, :],
                                         selap_t[off:off + 64, qq, :],
                                         dxy2[off:off + 64, rb, ci, :],
                                         start=True, stop=True)
                    rr = work4.tile([P, 2, N], F32, tag="fold_r")
                    nc.vector.tensor_scalar(rr[:], ups[:], MAGIC, -MAGIC,
                                            ALU.add, ALU.add)
                    ff = work4.tile([P, 2, N], F32, tag="fold_f")
                    nc.vector.tensor_tensor(ff[:], ups[:], rr[:],
                                            ALU.subtract)
                    # half-angle: s2 = sin(pi f), c2 = cos(pi f) = sin(pi/2-pi f)
                    s2 = work4.tile([P, 2, N], BF16, tag="s2")
                    nc.scalar.activation(s2[:], ff[:], AF.Sin, scale=PI_)
                    c2 = work4.tile([P, 2, N], BF16, tag="c2")
                    nc.scalar.activation(c2[:], ff[:], AF.Sin, scale=-PI_,
                                         bias=col_t[:, C_HPI:C_HPI + 1])
                    fsin = work4.tile([P, 2, N], BF16, tag="fsin")
                    nc.vector.tensor_tensor(fsin[:], s2[:], c2[:], ALU.mult)
                    fcos = work4.tile([P, 2, N], BF16, tag="fcos")
                    nc.vector.tensor_tensor(fcos[:], s2[:], s2[:], ALU.mult)
                    wgp = psum_wg.tile([P, N], F32, tag="wgp")
                    nc.tensor.matmul(wgp[:], wblk_b[:, 0, :], fsin[:, 0, :],
                                     start=True, stop=False)
                    nc.tensor.matmul(wgp[:], wblk_b[:, 1, :], fcos[:, 0, :],
                                     start=False, stop=False)
                    nc.tensor.matmul(wgp[:], wblk_b[:, 2, :], fsin[:, 1, :],
                                     start=False, stop=False)
                    nc.tensor.matmul(wgp[:], wblk_b[:, 3, :], fcos[:, 1, :],
                                     start=False, stop=False)
                    nc.tensor.matmul(wgp[:], lhs_wh[:], bankN[:],
                                     start=False, stop=True)
                    # wgdR = Relu(wg + bG' - eps)
                    nc.scalar.activation(wgd_il[:, g, :], wgp[:], AF.Relu,
                                         bias=col_t[:, C_BGR:C_BGR + 1])
                # bounce out: per-h writes [16, NG, N] -> dram [g, m, h, n]
                for h in range(H):
                    nc.sync.dma_start(
                        wgdd[rb, :, :, h, :].rearrange("g m n -> m g n"),
                        wgd_il[h * GM:(h + 1) * GM, :, :])

        # bounce in + uu precompute
        uu_all = persist.tile([P, NPAIR, NRB, 2, N], BF16, tag="uu_all")
        with tc.tile_pool(name="wstp", bufs=2) as wstp:
            for rb in range(NRB):
                wst = wstp.tile([P, H, N], BF16, tag="wst")
                nc.sync.dma_start(
                    wst[:], wgdd[rb].rearrange("g m h n -> (g m) h n"))
                for ob in range(NPAIR):
                    h0 = 2 * ob
                    u_ = wstp.tile([P, 2, N], BF16, tag="u_")
                    nc.vector.tensor_tensor(
                        u_[:], wst[:, h0:h0 + 2, :],
                        objpair[:, rb, None, :].to_broadcast((P, 2, N)),
                        ALU.mult)
                    nc.vector.tensor_tensor(
                        uu_all[:, ob, rb, :, :], u_[:],
                        objc[:, rb, None, :].to_broadcast((P, 2, N)),
                        ALU.add)

        # ---------------- phase 5: attention ----------------
        ot = persist.tile([P, NRB, N], BF16, tag="ot")
        with tc.tile_pool(name="work5", bufs=3) as work5, \
             tc.tile_pool(name="psum5", bufs=2, space="PSUM") as psum5, \
             tc.tile_pool(name="psum_s", bufs=1, space="PSUM") as psum_s, \
             tc.tile_pool(name="psum_av", bufs=1, space="PSUM") as psum_av, \
             tc.tile_pool(name="psum_rb", bufs=1, space="PSUM") as psum_rb:
            for ob in range(NPAIR):
                h0 = 2 * ob
                av = psum_av.tile([P, N], F32, tag="avps")
                sbank = psum_s.tile([2, N], F32, tag="sbank")
                for rb in range(NRB):
                    st2 = psum5.tile([P, 2, N], F32, tag="stps")
                    for hi in range(2):
                        po = hi * DK
                        nc.tensor.matmul(
                            st2[:, hi, :],
                            kTt[po:po + DK, ob, rb * P:(rb + 1) * P],
                            qT[po:po + DK, ob, :], start=True, stop=True)
                    e_ = work5.tile([P, 2, N], BF16, tag="e_t")
                    nc.scalar.activation(e_[:], st2[:], AF.Exp,
                                         bias=col_t[:, C_MC + rb:C_MC + rb + 1])
                    tt_ = work5.tile([P, 2, N], BF16, tag="tt_t")
                    nc.vector.tensor_tensor(
                        tt_[:], e_[:], uu_all[:, ob, rb, :, :], ALU.mult)
                    for hi in range(2):
                        po = hi * DK
                        nc.tensor.matmul(sbank[:], oh2_t[:, hi, :],
                                         tt_[:, hi, :],
                                         start=(rb == 0 and hi == 0),
                                         stop=(rb == NRB - 1 and hi == 1),
                                         skip_group_check=True)
                        nc.tensor.matmul(av[po:po + DK, :],
                                         v_sb[:, rb,
                                              (h0 + hi) * DK:(h0 + hi + 1) * DK],
                                         tt_[:, hi, :], start=(rb == 0),
                                         stop=(rb == NRB - 1),
                                         skip_group_check=True)
                rs = work5.tile([2, N], F32, tag="rs")
                nc.vector.reciprocal(rs[:], sbank[:])
                rrb = psum_rb.tile([P, N], F32, tag="rrb")
                nc.tensor.matmul(rrb[:], selp_f[:], rs[:],
                                 start=True, stop=True)
                av_sb = work5.tile([P, N], F32, tag="av_sb")
                nc.scalar.activation(av_sb[:], av[:], AF.Copy)
                nc.vector.tensor_tensor(ot[:, ob, :], av_sb[:], rrb[:],
                                        ALU.mult)

        # final projection: out[n, d]  (own PSUM scope)
        with tc.tile_pool(name="work6", bufs=2) as work6, \
             tc.tile_pool(name="psum6", bufs=2, space="PSUM") as psum6:
            for r in range(NRB):
                ps = psum6.tile([P, D], F32, tag="fps")
                for kt in range(NRB):
                    nc.tensor.matmul(ps[:], ot[:, kt, r * P:(r + 1) * P],
                                     wo_b[:, kt, :],
                                     start=(kt == 0), stop=(kt == NRB - 1))
                fo = work6.tile([P, D], F32, tag="fo")
                nc.vector.tensor_tensor(fo[:], ps[:], rowbc[:, R_BO, :],
                                        ALU.add)
                nc.sync.dma_start(out[r * P:(r + 1) * P, :], fo[:])

    _split_multi_waits(nc)
    return nc


_NC_CACHE = {}


def kernel(**inputs):
    in_maps = _host_prep(inputs)
    if "nc" not in _NC_CACHE:
        _NC_CACHE["nc"] = build_nc()
    nc = _NC_CACHE["nc"]
    res = run_bass_kernel_spmd(nc, in_maps, list(range(B)))
    out = np.stack([res.results[b]["out"] for b in range(B)], axis=0)
    return out.astype(np.float32)


if __name__ == "__main__":
    print("kernel module ok")
